# revision 13
# baseline (speedup 1.0000x reference)
"""BiLSTM-CRF NLL loss kernel for 8 Trainium2 NeuronCores (pure batch data-parallel).

Self-contained: hardcodes all shapes. Strategy per core (32 of 256 sequences):
  1. Host pre-gathers + transposes + augments the embeddings into embT
     [KAUG, TOK] bf16 (token-major, rows = emb dims + ones + (1-m)), streamed
     into SBUF end-ranges-first so the LSTM can start immediately.
  2. gx = Waug @ embT is computed by the PE directly INTO the per-step gates
     PSUM banks (ring of 2-step banks per direction, gate-major layout),
     a few steps ahead of the recurrence; the W_hh matmuls accumulate on
     top. No identity inject, no PSUM->SBUF copies.
  3. Two per-direction dependency chains (fwd t ascending, bwd descending),
     gates via tanh only (sigmoid(x)=0.5*tanh(x/2)+0.5), all loop DVE ops in
     bf16 (2x mode). Cell slot stores c2 = 2c; h~ = 2h with W_hh, W_fc
     pre-halved on host.
  4. fc GEMM + exp per 512-token chunk as soon as both h halves exist,
     interleaved into the loop's engine slack.
  5. CRF split in the middle: alpha runs t=0..95 forward, beta runs
     t=191..96 backward (invariant beta^T alpha), both chains concurrent,
     in scaled probability space (P scaled by e^-CCENT, renorm every 32).
  6. Gold score from host-built histograms/one-hots via PE reductions.
     Output: per-core sum of (fwd - gold); host divides by B.
"""

import numpy as np

import concourse.bacc as bacc
import concourse.bass as bass
import concourse.mybir as mybir
import concourse.tile as tile
from concourse import bass_utils

B, T, E, H2, V, NT = 256, 192, 300, 256, 11626, 13
H = H2 // 2          # 128
G4 = 4 * H           # 512
START, STOP = 0, 10
NCORES = 8
BC = B // NCORES     # 32 sequences per core
TOK = BC * T         # 6144 tokens per core, t-major: tok = t*BC + b
KAUG = E + 2         # emb dims + ones row + (1-m) row
KCH = [(0, 128), (128, 256), (256, KAUG)]   # K chunks of augmented GEMM
KPAIR = KAUG // 2    # 151 fp8 DoubleRow pairs
KCHD = [(0, 128), (128, KPAIR)]             # pair-space chunks for gx
RENORM = 32          # CRF renorm period
CCENT = 3.0
HM = T // 2          # 96: CRF meeting point
NREN = 2 * (HM // RENORM - 1)  # renorm events total (alpha + beta)
NCH = 12             # fc chunks (512 tokens each)
FCW = TOK // NCH     # 512
WS = 8               # steps per gates PSUM bank window

FP32 = mybir.dt.float32
BF16 = mybir.dt.bfloat16
FP8 = mybir.dt.float8e4
DR = mybir.MatmulPerfMode.DoubleRow
AF = mybir.ActivationFunctionType
ALU = mybir.AluOpType

_PROGRAM_CACHE = {}


def _emit(tc, io):
    nc = tc.nc
    embT = io["embT"]; waug = io["waug"]; whh = io["whh"]
    wfc = io["wfc"]; bfc = io["bfc"]
    transT = io["transT"]; trans = io["trans"]; pstop = io["pstop"]
    transflat = io["transflat"]
    oh = io["oh"]; pc = io["pc"]; out = io["out"]

    import contextlib
    ctx = contextlib.ExitStack()
    with ctx:
        consts = ctx.enter_context(tc.tile_pool(name="consts", bufs=1))

        # ---------- embT streamed in; embr = token blocks reversed ----------
        embT_sb = [consts.tile([k1 - k0, TOK], BF16, name=f"embT{i}")
                   for i, (k0, k1) in enumerate(KCH)]
        embR_sb = [consts.tile([k1 - k0, TOK], BF16, name=f"embR{i}")
                   for i, (k0, k1) in enumerate(KCH)]
        embr = io["embr"]
        NBLK = 4
        BW = TOK // NBLK  # 1536 tokens = 48 steps per superblock
        eng_ring = [nc.sync, nc.scalar, nc.gpsimd]
        ei = 0
        for bi in range(NBLK):
            c0, c1 = bi * BW, (bi + 1) * BW
            for ki, (k0, k1) in enumerate(KCH):
                eng = eng_ring[ei % 3]; ei += 1
                eng.dma_start(out=embT_sb[ki][:, c0:c1], in_=embT[k0:k1, c0:c1])
                eng = eng_ring[ei % 3]; ei += 1
                eng.dma_start(out=embR_sb[ki][:, c0:c1], in_=embr[k0:k1, c0:c1])

        # ---------- small constants ----------
        waug_sb = [consts.tile([k1 - k0, 2 * G4], BF16, name=f"waug{i}")
                   for i, (k0, k1) in enumerate(KCH)]
        for (k0, k1), t_ in zip(KCH, waug_sb):
            nc.sync.dma_start(out=t_[:], in_=waug[k0:k1, :])
        whh_sb = consts.tile([H, 2 * G4], BF16)
        nc.sync.dma_start(out=whh_sb[:], in_=whh[:])
        wfc_sb = consts.tile([H, 2 * NT], BF16)
        nc.sync.dma_start(out=wfc_sb[:, 0:NT], in_=wfc[0:H, :])
        nc.sync.dma_start(out=wfc_sb[:, NT:2 * NT], in_=wfc[H:H2, :])
        bfc_sb = consts.tile([NT, 1], FP32)
        nc.sync.dma_start(out=bfc_sb[:], in_=bfc[:])
        transT_sb = consts.tile([NT, NT], FP32)
        nc.sync.dma_start(out=transT_sb[:], in_=transT[:])
        trans_sb = consts.tile([NT, NT], FP32)
        nc.sync.dma_start(out=trans_sb[:], in_=trans[:])
        pstop_sb = consts.tile([1, NT], FP32)
        nc.sync.dma_start(out=pstop_sb[:], in_=pstop[:])
        tf_sb = consts.tile([128, 2], FP32)   # transflat split columns
        tfa = transflat[0:128]
        nc.gpsimd.dma_start(out=tf_sb[:, 0:1],
                            in_=bass.AP(tensor=tfa.tensor, offset=tfa.offset, ap=[[1, 128], [1, 1]]))
        tfb = transflat[128:169]
        nc.gpsimd.dma_start(out=tf_sb[0:41, 1:2],
                            in_=bass.AP(tensor=tfb.tensor, offset=tfb.offset, ap=[[1, 41], [1, 1]]))
        pc_sb = consts.tile([128, 2 * BC], FP32)
        nc.gpsimd.dma_start(out=pc_sb[:, 0:BC], in_=pc[0:128, :])
        nc.gpsimd.dma_start(out=pc_sb[0:41, BC:2 * BC], in_=pc[128:169, :])
        ones13_sb = consts.tile([NT, 1], FP32)
        nc.vector.memset(ones13_sb[:], 1.0)
        onesrow_sb = consts.tile([1, NT], FP32)
        nc.vector.memset(onesrow_sb[:], 1.0)
        negc_sb = consts.tile([NT, 1], FP32)
        nc.vector.memset(negc_sb[:], -CCENT)
        # alpha stationary: exp(transT - C); lhsT=exp(transT) -> applies P
        pts_sb = consts.tile([NT, NT], BF16)
        nc.scalar.activation(pts_sb[:], transT_sb[:], AF.Exp, bias=negc_sb[:, 0:1])
        # beta stationary: exp(trans - C); lhsT = P_s -> applies P_s^T
        pbs_sb = consts.tile([NT, NT], BF16)
        nc.scalar.activation(pbs_sb[:], trans_sb[:], AF.Exp, bias=negc_sb[:, 0:1])
        ones13b_sb = consts.tile([NT, 1], BF16)
        nc.vector.memset(ones13b_sb[:], 1.0)
        # pstop row exp (unscaled)
        pse_sb = consts.tile([1, NT], FP32)
        nc.scalar.activation(pse_sb[:], pstop_sb[:], AF.Exp)
        onescol_sb = consts.tile([1, BC], FP32)
        nc.vector.memset(onescol_sb[:], 1.0)

        # ---------- persistent loop state ----------
        hallp = ctx.enter_context(tc.tile_pool(name="hallp", bufs=1))
        h_all = [hallp.tile([H, TOK], BF16, name=f"hall{d}") for d in range(2)]
        late = ctx.enter_context(tc.tile_pool(name="late", bufs=1))
        ef = late.tile([NT, TOK], BF16)
        oh_sb = late.tile([NT, TOK], BF16)
        em_acc = [late.tile([NT, BC], FP32, name=f"emacc{i}") for i in range(2)]
        zbuf = late.tile([1, NREN * BC], FP32)
        nc.vector.memset(em_acc[0][:], 0.0)
        nc.gpsimd.dma_start(out=oh_sb[:], in_=oh[:])

        hinit = consts.tile([H, BC], BF16)
        nc.vector.memset(hinit[:], 0.0)
        emtp = ctx.enter_context(tc.tile_pool(name="emtp", bufs=1))
        emqp = ctx.enter_context(tc.tile_pool(name="emqp", bufs=2))

        # gates PSUM ring: 2 banks x [128, 8 slots x 8 steps x 32] fp32,
        # slot-major with slot = 2*gate + dir -> [i0 i1 f0 f1 o0 o1 g0 g1];
        # all 8 PSUM banks are used during the loop (fc runs post-loop).
        gbank = tc.alloc_tile_pool(name="gbank", bufs=2, space="PSUM")
        banks = [gbank.tile([H, 8 * WS * BC], FP32, name=f"bank{w}", tag="bank")
                 for w in range(2)]
        # T tiles ring of 2, shared across dirs:
        # [Ti0 Ti1 Tf0 Tf1 To0 To1 Tg0 Tg1 | c2_0 c2_1] (10 x 32), so the
        # gates tanh, q, c2' and cell tanh are each ONE instruction.
        tpool = tc.alloc_tile_pool(name="ltt", bufs=1)
        ttiles = [tpool.tile([H, 10 * BC], BF16, name=f"tt{r}") for r in range(2)]
        nc.vector.memset(ttiles[0][:, 8 * BC:10 * BC], 0.0)
        qpool = tc.alloc_tile_pool(name="lq", bufs=2)
        tcpool = tc.alloc_tile_pool(name="ltc", bufs=2)

        # ---------- gx emission (direct to PSUM banks) ----------
        def emit_gx1(d, w, g, ki):
            """One gx matmul: window w (steps 8w..8w+7), dir d, gate g, kchunk ki."""
            bk = banks[w % 2]
            k0, k1 = KCH[ki]
            kw = k1 - k0
            lhsT = waug_sb[ki][0:kw, d * G4 + g * H: d * G4 + (g + 1) * H]
            esb = (embT_sb if d == 0 else embR_sb)[ki]
            rhs = esb[0:kw, 8 * w * BC:(8 * w + WS) * BC]
            sd = 2 * g + d
            nc.tensor.matmul(bk[:, sd * WS * BC:(sd + 1) * WS * BC],
                             lhsT, rhs,
                             start=(ki == 0), stop=False,
                             skip_group_check=True)

        GX_ITEMS = [(d, g, ki) for d in range(2) for g in range(4)
                    for ki in range(3)]  # 24 per window, 3 per step

        # ---------- fc/emit ops (executed post-loop) ----------
        fc_state = {"nacc": 0}

        def fc_ops(k):
            cols = slice(k * FCW, (k + 1) * FCW)
            st = {}

            def op1():
                st["mm"] = fcp.tile([NT, FCW], FP32, tag="fcmm", name="fcmm")
                nc.tensor.matmul(st["mm"][:], wfc_sb[:, 0:NT],
                                 h_all[0][:, cols], start=True, stop=False,
                                 skip_group_check=True)

            def op2():
                nc.tensor.matmul(st["mm"][:], wfc_sb[:, NT:2 * NT],
                                 h_all[1][:, cols], start=False, stop=True,
                                 skip_group_check=True)

            def op3():
                c0 = k * FCW
                nc.scalar.activation(ef[:, c0:c0 + FCW // 2],
                                     st["mm"][:, 0:FCW // 2], AF.Exp,
                                     bias=bfc_sb[:, 0:1])

            def op4():
                c0 = k * FCW
                nc.scalar.activation(ef[:, c0 + FCW // 2:c0 + FCW],
                                     st["mm"][:, FCW // 2:FCW], AF.Exp,
                                     bias=bfc_sb[:, 0:1])

            def op5():
                st["emtmp"] = emtp.tile([NT, FCW], BF16, tag="emtmp", name="emtmp")
                nc.vector.scalar_tensor_tensor(
                    st["emtmp"][:], st["mm"][:], bfc_sb[:, 0:1], oh_sb[:, cols],
                    ALU.add, ALU.mult)

            def op6():
                st["emq"] = emqp.tile([NT, BC], FP32, tag="emq", name="emq")
                nc.vector.reduce_sum(
                    st["emq"][:],
                    st["emtmp"][:].rearrange("p (t b) -> p b t", t=FCW // BC),
                    axis=mybir.AxisListType.X)

            def op7():
                i = fc_state["nacc"]
                nc.gpsimd.tensor_tensor(em_acc[(i + 1) % 2][:], em_acc[i % 2][:],
                                        st["emq"][:], ALU.add)
                fc_state["nacc"] = i + 1

            return [op1, op2, op3, op4, op5, op6, op7]

        # ---------- LSTM step ----------
        def step_mms(d, s, h_prev_ap):
            bk = banks[(s // WS) % 2]
            par = s % WS
            for g in range(4):
                sd = 2 * g + d
                nc.tensor.matmul(
                    bk[:, sd * WS * BC + par * BC: sd * WS * BC + (par + 1) * BC],
                    whh_sb[:, d * G4 + g * H: d * G4 + (g + 1) * H],
                    h_prev_ap,
                    start=False, stop=True, skip_group_check=True)

        def step_gates(s):
            """merged gates tanh + q + c2' for both dirs."""
            bk = banks[(s // WS) % 2]
            par = s % WS
            gates_ap = bass.AP(tensor=bk.tensor, offset=bk[:, par * BC].offset,
                               ap=[bk.ap[0], [WS * BC, 8], [1, BC]])
            tt = ttiles[s % 2]
            tnx = ttiles[(s + 1) % 2]
            nc.scalar.activation(tt[:, 0:8 * BC], gates_ap, AF.Tanh, scale=0.5)
            # q = (T_if + 1) * [Tg | c2]: pairs i0<->g0, i1<->g1, f0<->c2_0, f1<->c2_1
            q = qpool.tile([H, 4 * BC], BF16, tag="q", name="q")
            nc.vector.scalar_tensor_tensor(
                q[:], tt[:, 0:4 * BC], 1.0, tt[:, 6 * BC:10 * BC],
                ALU.add, ALU.mult)
            # c2' = 0.5*qf + qi  ([qi0 qi1 qf0 qf1])
            nc.vector.scalar_tensor_tensor(
                tnx[:, 8 * BC:10 * BC], q[:, 2 * BC:4 * BC], 0.5, q[:, 0:2 * BC],
                ALU.mult, ALU.add)

        def step_cell(s):
            """merged cell tanh for both dirs."""
            tnx = ttiles[(s + 1) % 2]
            tc_ = tcpool.tile([H, 2 * BC], BF16, tag="tc", name="tc")
            nc.scalar.activation(tc_[:], tnx[:, 8 * BC:10 * BC], AF.Tanh, scale=0.5)
            return tc_

        def step_h(d, s, t, tc_):
            tt = ttiles[s % 2]
            hdst = h_all[d][:, t * BC:(t + 1) * BC]
            nc.vector.scalar_tensor_tensor(
                hdst, tt[:, (4 + d) * BC:(5 + d) * BC], 1.0, tc_[:, d * BC:(d + 1) * BC],
                ALU.add, ALU.mult)
            return hdst

        # ---------- main loop ----------
        # prologue: window 0 only; window w streams in during window w-1
        # (ring of 2 banks, 1-step WAR slack at boundaries)
        for (d, g, ki) in GX_ITEMS:
            emit_gx1(d, 0, g, ki)
        hp = [hinit[:], hinit[:]]
        for s in range(T):
            step_mms(0, s, hp[0])
            step_mms(1, s, hp[1])
            step_gates(s)
            tc_ = step_cell(s)
            hp[0] = step_h(0, s, s, tc_)
            hp[1] = step_h(1, s, T - 1 - s, tc_)
            w, j = s // WS + 1, s % WS
            if w < T // WS:
                for (d, g, ki) in GX_ITEMS[3 * j:3 * j + 3]:
                    emit_gx1(d, w, g, ki)

        tcpool.release()
        qpool.release()
        tpool.release()
        gbank.release()

        # ---------- fc + emit (post-loop, overlaps the CRF phase) ----------
        fcp = tc.alloc_tile_pool(name="fcp", bufs=2, space="PSUM")
        order = []
        lo_, hi_ = 0, NCH - 1
        while lo_ <= hi_:
            order.append(lo_); lo_ += 1
            if lo_ <= hi_:
                order.append(hi_); hi_ -= 1
        for k in order:
            for op in fc_ops(k):
                op()

        # ---------- CRF: alpha fwd 0..95, beta bwd 191..96 ----------
        apool = ctx.enter_context(tc.tile_pool(name="apool", bufs=3))
        bpool = ctx.enter_context(tc.tile_pool(name="bpool", bufs=3))
        crfp = tc.alloc_tile_pool(name="crfp", bufs=2, space="PSUM")
        crfz = tc.alloc_tile_pool(name="crfz", bufs=1, space="PSUM")
        zr = ctx.enter_context(tc.tile_pool(name="zr", bufs=2))
        efx = ctx.enter_context(tc.tile_pool(name="efx", bufs=2))

        A = apool.tile([NT, BC], BF16, tag="A", name="A")
        nc.vector.memset(A[:], 0.0)
        nc.vector.memset(A[START:START + 1, :], 1.0)
        # beta init: pstop broadcast over batch: pse^T @ ones[1,BC]
        Bt = crfp.tile([NT, BC], FP32, tag="rb", name="rb")
        nc.tensor.matmul(Bt[:], pse_sb[:], onescol_sb[:], start=True, stop=True,
                         skip_group_check=True)

        pend = [None, None]  # pre-scaled ef slices after renorm [alpha, beta]
        nren = [0]

        def renorm(X, which, tnext):
            """Fold 1/Z into the next step's ef slice; record Z. X must be SBUF."""
            k = nren[0]; nren[0] += 1
            zrow = crfz.tile([1, BC], FP32, tag="zrow", name="zrow")
            nc.tensor.matmul(zrow[:], ones13b_sb[:], X[:], start=True, stop=True,
                             skip_group_check=True)
            nc.scalar.copy(zbuf[:, k * BC:(k + 1) * BC], zrow[:])
            zrec = zr.tile([1, BC], FP32, tag=f"zrec{which}", name="zrec")
            nc.vector.reciprocal(zrec[:], zrow[:])
            zbc = crfz.tile([NT, BC], FP32, tag="zbc", name="zbc")
            nc.tensor.matmul(zbc[:], onesrow_sb[:], zrec[:], start=True, stop=True,
                             skip_group_check=True)
            nxt = efx.tile([NT, BC], BF16, tag=f"efx{which}", name="efx")
            cols = slice(tnext * BC, (tnext + 1) * BC)
            nc.vector.tensor_tensor(nxt[:], ef[:, cols], zbc[:], ALU.mult)
            pend[which] = nxt

        for i in range(HM):
            ta = i           # alpha consumes ef[ta]
            tb = T - 1 - i   # beta consumes ef[tb]
            # alpha: A <- (P@A) * ef[ta]
            r = crfp.tile([NT, BC], FP32, tag="ra", name="ra")
            nc.tensor.matmul(r[:], pts_sb[:], A[:], start=True, stop=True,
                             skip_group_check=True)
            A2 = apool.tile([NT, BC], BF16, tag="A", name="A")
            ef_ap = pend[0][:] if pend[0] is not None else ef[:, ta * BC:(ta + 1) * BC]
            pend[0] = None
            nc.vector.tensor_tensor(A2[:], r[:], ef_ap, ALU.mult)
            A = A2
            if (i + 1) % RENORM == 0 and i + 1 < HM:
                renorm(A, 0, ta + 1)
            # beta: B <- P^T @ (ef[tb] * B)   (Bt lives in PSUM except renorms)
            X = bpool.tile([NT, BC], BF16, tag="X", name="X")
            ef_bp = pend[1][:] if pend[1] is not None else ef[:, tb * BC:(tb + 1) * BC]
            pend[1] = None
            nc.vector.tensor_tensor(X[:], Bt[:], ef_bp, ALU.mult)
            B2 = crfp.tile([NT, BC], FP32, tag="rb", name="rb")
            nc.tensor.matmul(B2[:], pbs_sb[:], X[:], start=True, stop=True,
                             skip_group_check=True)
            Bt = B2
            if (i + 1) % RENORM == 0 and i + 1 < HM:
                Bs = bpool.tile([NT, BC], BF16, tag="Bs", name="Bs")
                nc.vector.tensor_copy(Bs[:], B2[:])
                Bt = Bs
                renorm(Bt, 1, tb - 1)

        # ---------- finals ----------
        fin = ctx.enter_context(tc.tile_pool(name="fin", bufs=1))
        # meet: fwd = ln(sum_j A[j]*B[j]) + sum ln Z
        meet = fin.tile([NT, BC], FP32)
        nc.vector.tensor_tensor(meet[:], A[:], Bt[:], ALU.mult)
        crfz.release()
        crfp.release()
        fcp.release()
        finp = ctx.enter_context(tc.tile_pool(name="finp", bufs=1, space="PSUM"))
        emred = em_acc[fc_state["nacc"] % 2]
        gold = finp.tile([1, BC], FP32)
        nc.tensor.matmul(gold[:], tf_sb[:, 0:1], pc_sb[:, 0:BC], start=True, stop=False,
                         skip_group_check=True)
        nc.tensor.matmul(gold[:], tf_sb[0:41, 1:2], pc_sb[0:41, BC:2 * BC],
                         start=False, stop=False, skip_group_check=True)
        nc.tensor.matmul(gold[:], ones13_sb[:], emred[:], start=False, stop=True,
                         skip_group_check=True)
        fmm = finp.tile([1, BC], FP32)
        nc.tensor.matmul(fmm[:], ones13_sb[:], meet[:], start=True, stop=True,
                         skip_group_check=True)
        lnz = fin.tile([1, NREN * BC], FP32)
        nc.scalar.activation(lnz[:], zbuf[:], AF.Ln)
        lsum = fin.tile([1, BC], FP32)
        nc.vector.reduce_sum(
            lsum[:], lnz[:].rearrange("p (k b) -> p b k", k=NREN),
            axis=mybir.AxisListType.X)
        lfin = fin.tile([1, BC], FP32)
        nc.scalar.activation(lfin[:], fmm[:], AF.Ln)
        fwd = fin.tile([1, BC], FP32)
        nc.vector.tensor_tensor(fwd[:], lfin[:], lsum[:], ALU.add)
        nll = fin.tile([1, BC], FP32)
        nc.vector.tensor_tensor(nll[:], fwd[:], gold[:], ALU.subtract)
        nllc = fin.tile([1, BC], FP32)
        nc.vector.tensor_scalar_add(nllc[:], nll[:], CCENT * T)
        tot = fin.tile([1, 1], FP32)
        nc.vector.reduce_sum(tot[:], nllc[:], axis=mybir.AxisListType.X)
        nc.sync.dma_start(out=out[:], in_=tot[:])


def build_program():
    key = "nc"
    if key in _PROGRAM_CACHE:
        return _PROGRAM_CACHE[key]
    nc = bacc.Bacc("TRN2", target_bir_lowering=False, debug=False, num_devices=NCORES)
    io = {
        "embT": nc.dram_tensor("embT", [KAUG, TOK], BF16, kind="ExternalInput").ap(),
        "embr": nc.dram_tensor("embr", [KAUG, TOK], BF16, kind="ExternalInput").ap(),
        "waug": nc.dram_tensor("waug", [KAUG, 2 * G4], BF16, kind="ExternalInput").ap(),
        "whh": nc.dram_tensor("whh", [H, 2 * G4], BF16, kind="ExternalInput").ap(),
        "wfc": nc.dram_tensor("wfc", [H2, NT], BF16, kind="ExternalInput").ap(),
        "bfc": nc.dram_tensor("bfc", [NT, 1], FP32, kind="ExternalInput").ap(),
        "transT": nc.dram_tensor("transT", [NT, NT], FP32, kind="ExternalInput").ap(),
        "trans": nc.dram_tensor("trans", [NT, NT], FP32, kind="ExternalInput").ap(),
        "pstop": nc.dram_tensor("pstop", [1, NT], FP32, kind="ExternalInput").ap(),
        "transflat": nc.dram_tensor("transflat", [NT * NT], FP32, kind="ExternalInput").ap(),
        "oh": nc.dram_tensor("oh", [NT, TOK], BF16, kind="ExternalInput").ap(),
        "pc": nc.dram_tensor("pc", [NT * NT, BC], FP32, kind="ExternalInput").ap(),
        "out": nc.dram_tensor("out", [1, 1], FP32, kind="ExternalOutput").ap(),
    }
    with tile.TileContext(nc) as tc:
        _emit(tc, io)
    nc.compile()
    _PROGRAM_CACHE[key] = nc
    return nc


def host_prep(inputs):
    """Build the 8 per-core input maps (host does index/layout/dtype prep)."""
    import ml_dtypes
    bf16 = ml_dtypes.bfloat16
    fp8 = ml_dtypes.float8_e4m3

    def to_fp8(x):
        return np.clip(x, -240.0, 240.0).astype(fp8)

    sent = np.asarray(inputs["sentence"]).astype(np.int64)        # [B,T]
    seq_len = np.asarray(inputs["seq_len"]).astype(np.int64)
    tags = np.asarray(inputs["tags"]).astype(np.int64)            # [B,T]
    lens = np.clip(seq_len, 1, T)
    mask = (np.arange(T)[None, :] < lens[:, None]).astype(np.float32)  # [B,T]
    embtab = np.asarray(inputs["embedding"], np.float32)

    def reorder(Wx):  # pytorch gate order i,f,g,o -> i,f,o,g
        i, f, g, o = np.split(np.asarray(Wx, np.float32), 4, 0)
        return np.concatenate([i, f, o, g], 0)

    def build_waug(W_ih, bvec, is_bwd):
        Wr = reorder(W_ih).copy()   # [4H, E]
        br = reorder(np.asarray(bvec, np.float32)[:, None])[:, 0].copy()
        Wr[3 * H:4 * H] *= 2.0      # g-gate preact x2: tanh(0.5*(2x)) = tanh(x)
        br[3 * H:4 * H] *= 2.0
        Waug = np.zeros((KAUG, G4), np.float32)
        Waug[0:E, :] = Wr.T
        Waug[E, :] = br             # ones row -> bias
        if is_bwd:
            Waug[E + 1, 0:2 * H] = -1e9  # (1-m) row -> i,f preact mask (freeze c=0)
        else:
            Waug[E + 1, 2 * H:3 * H] = -1e9  # o-gate mask: zero padded fwd outputs
        return Waug

    waug = np.concatenate(
        [build_waug(inputs["W_ih_f"], inputs["b_f"], False),
         build_waug(inputs["W_ih_b"], inputs["b_b"], True)], axis=1
    ).astype(bf16)                                               # [KAUG, 1024]

    def whh_prep(W):
        Wr = reorder(W).copy()
        Wr[0:3 * H] *= 0.5          # rhs is h~ = 2h
        return Wr.T
    whh = np.concatenate(
        [whh_prep(inputs["W_hh_f"]), whh_prep(inputs["W_hh_b"])], axis=1
    ).astype(np.float32).astype(bf16)                            # [H, 1024]
    wfc = np.ascontiguousarray(
        0.5 * np.asarray(inputs["W_fc"], np.float32).T).astype(bf16)  # [H2,NT]
    bfc = np.asarray(inputs["b_fc"], np.float32).reshape(NT, 1)
    trans = np.asarray(inputs["transitions"], np.float32)
    transT = np.ascontiguousarray(trans.T)
    transflat = np.ascontiguousarray(trans.reshape(-1))
    pstop = np.ascontiguousarray(trans[STOP, :].reshape(1, NT))

    in_maps = []
    for core in range(NCORES):
        sl = slice(core * BC, (core + 1) * BC)
        s_c, t_c, m_c = sent[sl], tags[sl], mask[sl]             # [BC,T]
        # embT: [KAUG, TOK] token-major (tok = t*BC + b);
        # embr = token blocks reversed (t -> T-1-t) for the bwd direction
        emb_g = embtab[s_c]                                      # [BC,T,E]
        embT_f = np.empty((KAUG, TOK), dtype=np.float32)
        embT_f[0:E, :] = emb_g.transpose(2, 1, 0).reshape(E, TOK)
        embT_f[E, :] = 1.0
        embT_f[E + 1, :] = 1.0 - m_c.T.reshape(-1)
        embT_m = embT_f.astype(bf16)
        embr_m = embT_f.reshape(KAUG, T, BC)[:, ::-1, :].reshape(KAUG, TOK).astype(bf16)
        # one-hot [NT, TOK]
        ohm = np.zeros((NT, TOK), np.float32)
        ttm = t_c.T.reshape(-1)
        ohm[ttm, np.arange(TOK)] = 1.0
        ohm = ohm.astype(bf16)
        # pair-count histogram [169, BC] incl STOP term
        pcm = np.zeros((NT * NT, BC), np.float32)
        text = np.concatenate([np.full((BC, 1), START, np.int64), t_c], 1)
        for b_ in range(BC):
            idx = text[b_, 1:] * NT + text[b_, :-1]
            np.add.at(pcm[:, b_], idx, 1.0)
            pcm[STOP * NT + t_c[b_, -1], b_] += 1.0
        in_maps.append({
            "embT": embT_m, "embr": embr_m, "waug": waug, "whh": whh, "wfc": wfc, "bfc": bfc,
            "transT": transT, "trans": trans, "pstop": pstop,
            "transflat": transflat, "oh": ohm, "pc": pcm,
        })
    return in_maps


def kernel(**inputs):
    nc = build_program()
    in_maps = host_prep(inputs)
    res = bass_utils.run_bass_kernel_spmd(nc, in_maps, list(range(NCORES)))
    total = sum(float(r["out"][0, 0]) for r in res.results)
    return np.float32(total / B)


# revision 15
# speedup vs baseline: 1.1947x; 1.1947x over previous
"""BiLSTM-CRF NLL loss kernel for 8 Trainium2 NeuronCores (pure batch data-parallel).

Self-contained: hardcodes all shapes. Strategy per core (32 of 256 sequences):
  1. Host pre-gathers + transposes + augments the embeddings into embT
     [KAUG, TOK] bf16 (token-major, rows = emb dims + ones + (1-m)), streamed
     into SBUF end-ranges-first so the LSTM can start immediately.
  2. gx = Waug @ embT is computed by the PE directly INTO the per-step gates
     PSUM banks (ring of 2-step banks per direction, gate-major layout),
     a few steps ahead of the recurrence; the W_hh matmuls accumulate on
     top. No identity inject, no PSUM->SBUF copies.
  3. Two per-direction dependency chains (fwd t ascending, bwd descending),
     gates via tanh only (sigmoid(x)=0.5*tanh(x/2)+0.5), all loop DVE ops in
     bf16 (2x mode). Cell slot stores c2 = 2c; h~ = 2h with W_hh, W_fc
     pre-halved on host.
  4. fc GEMM + exp per 512-token chunk as soon as both h halves exist,
     interleaved into the loop's engine slack.
  5. CRF split in the middle: alpha runs t=0..95 forward, beta runs
     t=191..96 backward (invariant beta^T alpha), both chains concurrent,
     in scaled probability space (P scaled by e^-CCENT, renorm every 32).
  6. Gold score from host-built histograms/one-hots via PE reductions.
     Output: per-core sum of (fwd - gold); host divides by B.
"""

import numpy as np

import concourse.bacc as bacc
import concourse.bass as bass
import concourse.mybir as mybir
import concourse.tile as tile
from concourse import bass_utils

B, T, E, H2, V, NT = 256, 192, 300, 256, 11626, 13
H = H2 // 2          # 128
G4 = 4 * H           # 512
START, STOP = 0, 10
NCORES = 8
BC = B // NCORES     # 32 sequences per core
TOK = BC * T         # 6144 tokens per core, t-major: tok = t*BC + b
KAUG = E + 2         # emb dims + ones row + (1-m) row
KCH = [(0, 128), (128, 256), (256, KAUG)]   # K chunks of augmented GEMM
KPAIR = KAUG // 2    # 151 fp8 DoubleRow pairs
KCHD = [(0, 128), (128, KPAIR)]             # pair-space chunks for gx
RENORM = 32          # CRF renorm period
CCENT = 3.0
HM = T // 2          # 96: CRF meeting point
NREN = 2 * (HM // RENORM - 1)  # renorm events total (alpha + beta)
NCH = 12             # fc chunks (512 tokens each)
FCW = TOK // NCH     # 512
WS = 8               # steps per gates PSUM bank window

FP32 = mybir.dt.float32
BF16 = mybir.dt.bfloat16
FP8 = mybir.dt.float8e4
DR = mybir.MatmulPerfMode.DoubleRow
AF = mybir.ActivationFunctionType
ALU = mybir.AluOpType

_PROGRAM_CACHE = {}


def _emit(tc, io):
    nc = tc.nc
    embT = io["embT"]; waug = io["waug"]; whh = io["whh"]
    wfc = io["wfc"]; bfc = io["bfc"]
    transT = io["transT"]; trans = io["trans"]; pstop = io["pstop"]
    transflat = io["transflat"]
    oh = io["oh"]; pc = io["pc"]; out = io["out"]

    import contextlib
    ctx = contextlib.ExitStack()
    with ctx:
        consts = ctx.enter_context(tc.tile_pool(name="consts", bufs=1))

        # ---------- embT streamed in; embr = token blocks reversed ----------
        embT_sb = [consts.tile([k1 - k0, TOK], BF16, name=f"embT{i}")
                   for i, (k0, k1) in enumerate(KCH)]
        embR_sb = [consts.tile([k1 - k0, TOK], BF16, name=f"embR{i}")
                   for i, (k0, k1) in enumerate(KCH)]
        embr = io["embr"]
        NBLK = 4
        BW = TOK // NBLK  # 1536 tokens = 48 steps per superblock
        eng_ring = [nc.sync, nc.scalar, nc.gpsimd]
        ei = 0
        for bi in range(NBLK):
            c0, c1 = bi * BW, (bi + 1) * BW
            for ki, (k0, k1) in enumerate(KCH):
                eng = eng_ring[ei % 3]; ei += 1
                eng.dma_start(out=embT_sb[ki][:, c0:c1], in_=embT[k0:k1, c0:c1])
                eng = eng_ring[ei % 3]; ei += 1
                eng.dma_start(out=embR_sb[ki][:, c0:c1], in_=embr[k0:k1, c0:c1])

        # ---------- small constants ----------
        waug_sb = [consts.tile([k1 - k0, 2 * G4], BF16, name=f"waug{i}")
                   for i, (k0, k1) in enumerate(KCH)]
        for (k0, k1), t_ in zip(KCH, waug_sb):
            nc.sync.dma_start(out=t_[:], in_=waug[k0:k1, :])
        whh_sb = consts.tile([H, 2 * G4], BF16)
        nc.sync.dma_start(out=whh_sb[:], in_=whh[:])
        wfc_sb = consts.tile([H, 2 * NT], BF16)
        nc.sync.dma_start(out=wfc_sb[:, 0:NT], in_=wfc[0:H, :])
        nc.sync.dma_start(out=wfc_sb[:, NT:2 * NT], in_=wfc[H:H2, :])
        bfc_sb = consts.tile([NT, 1], FP32)
        nc.sync.dma_start(out=bfc_sb[:], in_=bfc[:])
        transT_sb = consts.tile([NT, NT], FP32)
        nc.sync.dma_start(out=transT_sb[:], in_=transT[:])
        trans_sb = consts.tile([NT, NT], FP32)
        nc.sync.dma_start(out=trans_sb[:], in_=trans[:])
        pstop_sb = consts.tile([1, NT], FP32)
        nc.sync.dma_start(out=pstop_sb[:], in_=pstop[:])
        tf_sb = consts.tile([128, 2], FP32)   # transflat split columns
        tfa = transflat[0:128]
        nc.gpsimd.dma_start(out=tf_sb[:, 0:1],
                            in_=bass.AP(tensor=tfa.tensor, offset=tfa.offset, ap=[[1, 128], [1, 1]]))
        tfb = transflat[128:169]
        nc.gpsimd.dma_start(out=tf_sb[0:41, 1:2],
                            in_=bass.AP(tensor=tfb.tensor, offset=tfb.offset, ap=[[1, 41], [1, 1]]))
        pc_sb = consts.tile([128, 2 * BC], FP32)
        nc.gpsimd.dma_start(out=pc_sb[:, 0:BC], in_=pc[0:128, :])
        nc.gpsimd.dma_start(out=pc_sb[0:41, BC:2 * BC], in_=pc[128:169, :])
        ones13_sb = consts.tile([NT, 1], FP32)
        nc.vector.memset(ones13_sb[:], 1.0)
        onesrow_sb = consts.tile([1, NT], FP32)
        nc.vector.memset(onesrow_sb[:], 1.0)
        negc_sb = consts.tile([NT, 1], FP32)
        nc.vector.memset(negc_sb[:], -CCENT)
        # alpha stationary: exp(transT - C); lhsT=exp(transT) -> applies P
        pts_sb = consts.tile([NT, NT], BF16)
        nc.scalar.activation(pts_sb[:], transT_sb[:], AF.Exp, bias=negc_sb[:, 0:1])
        # beta stationary: exp(trans - C); lhsT = P_s -> applies P_s^T
        pbs_sb = consts.tile([NT, NT], BF16)
        nc.scalar.activation(pbs_sb[:], trans_sb[:], AF.Exp, bias=negc_sb[:, 0:1])
        ones13b_sb = consts.tile([NT, 1], BF16)
        nc.vector.memset(ones13b_sb[:], 1.0)
        # pstop row exp (unscaled)
        pse_sb = consts.tile([1, NT], FP32)
        nc.scalar.activation(pse_sb[:], pstop_sb[:], AF.Exp)
        onescol_sb = consts.tile([1, BC], FP32)
        nc.vector.memset(onescol_sb[:], 1.0)

        # ---------- persistent loop state ----------
        hallp = ctx.enter_context(tc.tile_pool(name="hallp", bufs=1))
        h_all = [hallp.tile([H, TOK], BF16, name=f"hall{d}") for d in range(2)]
        late = ctx.enter_context(tc.tile_pool(name="late", bufs=1))
        ef = late.tile([NT, TOK], BF16)
        oh_sb = late.tile([NT, TOK], BF16)
        em_acc = [late.tile([NT, BC], FP32, name=f"emacc{i}") for i in range(2)]
        zbuf = late.tile([1, NREN * BC], FP32)
        nc.vector.memset(em_acc[0][:], 0.0)
        nc.gpsimd.dma_start(out=oh_sb[:], in_=oh[:])

        hinit = consts.tile([H, BC], BF16)
        nc.vector.memset(hinit[:], 0.0)
        emtp = ctx.enter_context(tc.tile_pool(name="emtp", bufs=1))
        emqp = ctx.enter_context(tc.tile_pool(name="emqp", bufs=2))

        # gates PSUM ring: per dir, 2 banks x [128, 4 gates x 8 steps x 32]
        # fp32 (gate-major: gate g at cols [g*8BC + (s%8)*BC]); all 8 PSUM
        # banks are used during the loop (fc runs post-loop).
        gbank = tc.alloc_tile_pool(name="gbank", bufs=2, space="PSUM")
        banks = [[gbank.tile([H, WS * 4 * BC], FP32, name=f"bank{d}{w}", tag=f"bank{d}")
                  for w in range(2)] for d in range(2)]
        # T tiles: ring of 2; each ring tile holds both dirs at base d*5BC
        # with per-dir layout [Ti|Tf|To|Tg|c2], so q/c2 stay per-dir
        # contiguous while the cell tanh merges both dirs in one ACT op.
        tpool = tc.alloc_tile_pool(name="ltt", bufs=1)
        ttiles = [tpool.tile([H, 2 * 5 * BC], BF16, name=f"tt{r}") for r in range(2)]
        for d in range(2):
            nc.vector.memset(ttiles[0][:, d * 5 * BC + 4 * BC:(d + 1) * 5 * BC], 0.0)
        qpool = tc.alloc_tile_pool(name="lq", bufs=2)
        tcpool = tc.alloc_tile_pool(name="ltc", bufs=2)

        # ---------- gx emission (direct to PSUM banks) ----------
        def emit_gx1(d, w, g, ki):
            """One gx matmul: window w (steps 8w..8w+7), dir d, gate g, kchunk ki."""
            bk = banks[d][w % 2]
            k0, k1 = KCH[ki]
            kw = k1 - k0
            lhsT = waug_sb[ki][0:kw, d * G4 + g * H: d * G4 + (g + 1) * H]
            esb = (embT_sb if d == 0 else embR_sb)[ki]
            rhs = esb[0:kw, 8 * w * BC:(8 * w + WS) * BC]
            nc.tensor.matmul(bk[:, g * WS * BC:(g + 1) * WS * BC],
                             lhsT, rhs,
                             start=(ki == 0), stop=False,
                             skip_group_check=True)

        GX_ITEMS = [(d, g, ki) for d in range(2) for g in range(4)
                    for ki in range(3)]  # 24 per window, 3 per step

        # ---------- fc/emit ops (executed post-loop) ----------
        fc_state = {"nacc": 0}

        def fc_ops(k):
            cols = slice(k * FCW, (k + 1) * FCW)
            st = {}

            def op1():
                st["mm"] = fcp.tile([NT, FCW], FP32, tag="fcmm", name="fcmm")
                nc.tensor.matmul(st["mm"][:], wfc_sb[:, 0:NT],
                                 h_all[0][:, cols], start=True, stop=False,
                                 skip_group_check=True)

            def op2():
                nc.tensor.matmul(st["mm"][:], wfc_sb[:, NT:2 * NT],
                                 h_all[1][:, cols], start=False, stop=True,
                                 skip_group_check=True)

            def op3():
                c0 = k * FCW
                nc.scalar.activation(ef[:, c0:c0 + FCW // 2],
                                     st["mm"][:, 0:FCW // 2], AF.Exp,
                                     bias=bfc_sb[:, 0:1])

            def op4():
                c0 = k * FCW
                nc.scalar.activation(ef[:, c0 + FCW // 2:c0 + FCW],
                                     st["mm"][:, FCW // 2:FCW], AF.Exp,
                                     bias=bfc_sb[:, 0:1])

            def op5():
                st["emtmp"] = emtp.tile([NT, FCW], BF16, tag="emtmp", name="emtmp")
                nc.vector.scalar_tensor_tensor(
                    st["emtmp"][:], st["mm"][:], bfc_sb[:, 0:1], oh_sb[:, cols],
                    ALU.add, ALU.mult)

            def op6():
                st["emq"] = emqp.tile([NT, BC], FP32, tag="emq", name="emq")
                nc.vector.reduce_sum(
                    st["emq"][:],
                    st["emtmp"][:].rearrange("p (t b) -> p b t", t=FCW // BC),
                    axis=mybir.AxisListType.X)

            def op7():
                i = fc_state["nacc"]
                nc.gpsimd.tensor_tensor(em_acc[(i + 1) % 2][:], em_acc[i % 2][:],
                                        st["emq"][:], ALU.add)
                fc_state["nacc"] = i + 1

            return [op1, op2, op3, op4, op5, op6, op7]

        # ---------- LSTM step ----------
        def step_mms(d, s, h_prev_ap):
            bk = banks[d][(s // WS) % 2]
            par = s % WS
            for g in range(4):
                nc.tensor.matmul(
                    bk[:, g * WS * BC + par * BC: g * WS * BC + (par + 1) * BC],
                    whh_sb[:, d * G4 + g * H: d * G4 + (g + 1) * H],
                    h_prev_ap,
                    start=False, stop=True, skip_group_check=True)

        def step_gates(d, s):
            """per-dir gates tanh + q + c2'."""
            bk = banks[d][(s // WS) % 2]
            par = s % WS
            b0 = d * 5 * BC
            gates_ap = bass.AP(tensor=bk.tensor, offset=bk[:, par * BC].offset,
                               ap=[bk.ap[0], [WS * BC, 4], [1, BC]])
            tt = ttiles[s % 2]
            tnx = ttiles[(s + 1) % 2]
            nc.scalar.activation(tt[:, b0:b0 + 4 * BC], gates_ap, AF.Tanh, scale=0.5)
            q = qpool.tile([H, 2 * BC], BF16, tag=f"q{d}", name=f"q{d}")
            nc.vector.scalar_tensor_tensor(
                q[:], tt[:, b0:b0 + 2 * BC], 1.0, tt[:, b0 + 3 * BC:b0 + 5 * BC],
                ALU.add, ALU.mult)
            nc.vector.scalar_tensor_tensor(
                tnx[:, b0 + 4 * BC:b0 + 5 * BC], q[:, BC:2 * BC], 0.5, q[:, 0:BC],
                ALU.mult, ALU.add)

        def step_cell(s):
            """merged cell tanh for both dirs."""
            tnx = ttiles[(s + 1) % 2]
            cin = bass.AP(tensor=tnx.tensor, offset=tnx[:, 4 * BC].offset,
                          ap=[tnx.ap[0], [5 * BC, 2], [1, BC]])
            tc_ = tcpool.tile([H, 2 * BC], BF16, tag="tc", name="tc")
            nc.scalar.activation(tc_[:], cin, AF.Tanh, scale=0.5)
            return tc_

        def step_h(d, s, t, tc_):
            tt = ttiles[s % 2]
            b0 = d * 5 * BC
            hdst = h_all[d][:, t * BC:(t + 1) * BC]
            nc.vector.scalar_tensor_tensor(
                hdst, tt[:, b0 + 2 * BC:b0 + 3 * BC], 1.0, tc_[:, d * BC:(d + 1) * BC],
                ALU.add, ALU.mult)
            return hdst

        # ---------- main loop ----------
        # prologue: window 0 only; window w streams in during window w-1
        # (ring of 2 banks, 1-step WAR slack at boundaries)
        for (d, g, ki) in GX_ITEMS:
            emit_gx1(d, 0, g, ki)
        hp = [hinit[:], hinit[:]]
        for s in range(T):
            step_mms(0, s, hp[0])
            step_mms(1, s, hp[1])
            step_gates(0, s)
            step_gates(1, s)
            tc_ = step_cell(s)
            hp[0] = step_h(0, s, s, tc_)
            hp[1] = step_h(1, s, T - 1 - s, tc_)
            w, j = s // WS + 1, s % WS
            if w < T // WS:
                for (d, g, ki) in GX_ITEMS[3 * j:3 * j + 3]:
                    emit_gx1(d, w, g, ki)

        tcpool.release()
        qpool.release()
        tpool.release()
        gbank.release()

        # ---------- fc + emit (post-loop, overlaps the CRF phase) ----------
        fcp = tc.alloc_tile_pool(name="fcp", bufs=2, space="PSUM")
        order = []
        lo_, hi_ = 0, NCH - 1
        while lo_ <= hi_:
            order.append(lo_); lo_ += 1
            if lo_ <= hi_:
                order.append(hi_); hi_ -= 1
        for k in order:
            for op in fc_ops(k):
                op()

        # ---------- CRF: alpha fwd 0..95, beta bwd 191..96 ----------
        apool = ctx.enter_context(tc.tile_pool(name="apool", bufs=3))
        bpool = ctx.enter_context(tc.tile_pool(name="bpool", bufs=3))
        crfp = tc.alloc_tile_pool(name="crfp", bufs=2, space="PSUM")
        crfz = tc.alloc_tile_pool(name="crfz", bufs=1, space="PSUM")
        zr = ctx.enter_context(tc.tile_pool(name="zr", bufs=2))
        efx = ctx.enter_context(tc.tile_pool(name="efx", bufs=2))

        A = apool.tile([NT, BC], BF16, tag="A", name="A")
        nc.vector.memset(A[:], 0.0)
        nc.vector.memset(A[START:START + 1, :], 1.0)
        # beta init: pstop broadcast over batch: pse^T @ ones[1,BC]
        Bt = crfp.tile([NT, BC], FP32, tag="rb", name="rb")
        nc.tensor.matmul(Bt[:], pse_sb[:], onescol_sb[:], start=True, stop=True,
                         skip_group_check=True)

        pend = [None, None]  # pre-scaled ef slices after renorm [alpha, beta]
        nren = [0]

        def renorm(X, which, tnext):
            """Fold 1/Z into the next step's ef slice; record Z. X must be SBUF."""
            k = nren[0]; nren[0] += 1
            zrow = crfz.tile([1, BC], FP32, tag="zrow", name="zrow")
            nc.tensor.matmul(zrow[:], ones13b_sb[:], X[:], start=True, stop=True,
                             skip_group_check=True)
            nc.scalar.copy(zbuf[:, k * BC:(k + 1) * BC], zrow[:])
            zrec = zr.tile([1, BC], FP32, tag=f"zrec{which}", name="zrec")
            nc.vector.reciprocal(zrec[:], zrow[:])
            zbc = crfz.tile([NT, BC], FP32, tag="zbc", name="zbc")
            nc.tensor.matmul(zbc[:], onesrow_sb[:], zrec[:], start=True, stop=True,
                             skip_group_check=True)
            nxt = efx.tile([NT, BC], BF16, tag=f"efx{which}", name="efx")
            cols = slice(tnext * BC, (tnext + 1) * BC)
            nc.vector.tensor_tensor(nxt[:], ef[:, cols], zbc[:], ALU.mult)
            pend[which] = nxt

        for i in range(HM):
            ta = i           # alpha consumes ef[ta]
            tb = T - 1 - i   # beta consumes ef[tb]
            # alpha: A <- (P@A) * ef[ta]
            r = crfp.tile([NT, BC], FP32, tag="ra", name="ra")
            nc.tensor.matmul(r[:], pts_sb[:], A[:], start=True, stop=True,
                             skip_group_check=True)
            A2 = apool.tile([NT, BC], BF16, tag="A", name="A")
            ef_ap = pend[0][:] if pend[0] is not None else ef[:, ta * BC:(ta + 1) * BC]
            pend[0] = None
            nc.vector.tensor_tensor(A2[:], r[:], ef_ap, ALU.mult)
            A = A2
            if (i + 1) % RENORM == 0 and i + 1 < HM:
                renorm(A, 0, ta + 1)
            # beta: B <- P^T @ (ef[tb] * B)   (Bt lives in PSUM except renorms)
            X = bpool.tile([NT, BC], BF16, tag="X", name="X")
            ef_bp = pend[1][:] if pend[1] is not None else ef[:, tb * BC:(tb + 1) * BC]
            pend[1] = None
            nc.vector.tensor_tensor(X[:], Bt[:], ef_bp, ALU.mult)
            B2 = crfp.tile([NT, BC], FP32, tag="rb", name="rb")
            nc.tensor.matmul(B2[:], pbs_sb[:], X[:], start=True, stop=True,
                             skip_group_check=True)
            Bt = B2
            if (i + 1) % RENORM == 0 and i + 1 < HM:
                Bs = bpool.tile([NT, BC], BF16, tag="Bs", name="Bs")
                nc.vector.tensor_copy(Bs[:], B2[:])
                Bt = Bs
                renorm(Bt, 1, tb - 1)

        # ---------- finals ----------
        fin = ctx.enter_context(tc.tile_pool(name="fin", bufs=1))
        # meet: fwd = ln(sum_j A[j]*B[j]) + sum ln Z
        meet = fin.tile([NT, BC], FP32)
        nc.vector.tensor_tensor(meet[:], A[:], Bt[:], ALU.mult)
        crfz.release()
        crfp.release()
        fcp.release()
        finp = ctx.enter_context(tc.tile_pool(name="finp", bufs=1, space="PSUM"))
        emred = em_acc[fc_state["nacc"] % 2]
        gold = finp.tile([1, BC], FP32)
        nc.tensor.matmul(gold[:], tf_sb[:, 0:1], pc_sb[:, 0:BC], start=True, stop=False,
                         skip_group_check=True)
        nc.tensor.matmul(gold[:], tf_sb[0:41, 1:2], pc_sb[0:41, BC:2 * BC],
                         start=False, stop=False, skip_group_check=True)
        nc.tensor.matmul(gold[:], ones13_sb[:], emred[:], start=False, stop=True,
                         skip_group_check=True)
        fmm = finp.tile([1, BC], FP32)
        nc.tensor.matmul(fmm[:], ones13_sb[:], meet[:], start=True, stop=True,
                         skip_group_check=True)
        lnz = fin.tile([1, NREN * BC], FP32)
        nc.scalar.activation(lnz[:], zbuf[:], AF.Ln)
        lsum = fin.tile([1, BC], FP32)
        nc.vector.reduce_sum(
            lsum[:], lnz[:].rearrange("p (k b) -> p b k", k=NREN),
            axis=mybir.AxisListType.X)
        lfin = fin.tile([1, BC], FP32)
        nc.scalar.activation(lfin[:], fmm[:], AF.Ln)
        fwd = fin.tile([1, BC], FP32)
        nc.vector.tensor_tensor(fwd[:], lfin[:], lsum[:], ALU.add)
        nll = fin.tile([1, BC], FP32)
        nc.vector.tensor_tensor(nll[:], fwd[:], gold[:], ALU.subtract)
        nllc = fin.tile([1, BC], FP32)
        nc.vector.tensor_scalar_add(nllc[:], nll[:], CCENT * T)
        tot = fin.tile([1, 1], FP32)
        nc.vector.reduce_sum(tot[:], nllc[:], axis=mybir.AxisListType.X)
        nc.sync.dma_start(out=out[:], in_=tot[:])


def build_program():
    key = "nc"
    if key in _PROGRAM_CACHE:
        return _PROGRAM_CACHE[key]
    nc = bacc.Bacc("TRN2", target_bir_lowering=False, debug=False, num_devices=NCORES)
    io = {
        "embT": nc.dram_tensor("embT", [KAUG, TOK], BF16, kind="ExternalInput").ap(),
        "embr": nc.dram_tensor("embr", [KAUG, TOK], BF16, kind="ExternalInput").ap(),
        "waug": nc.dram_tensor("waug", [KAUG, 2 * G4], BF16, kind="ExternalInput").ap(),
        "whh": nc.dram_tensor("whh", [H, 2 * G4], BF16, kind="ExternalInput").ap(),
        "wfc": nc.dram_tensor("wfc", [H2, NT], BF16, kind="ExternalInput").ap(),
        "bfc": nc.dram_tensor("bfc", [NT, 1], FP32, kind="ExternalInput").ap(),
        "transT": nc.dram_tensor("transT", [NT, NT], FP32, kind="ExternalInput").ap(),
        "trans": nc.dram_tensor("trans", [NT, NT], FP32, kind="ExternalInput").ap(),
        "pstop": nc.dram_tensor("pstop", [1, NT], FP32, kind="ExternalInput").ap(),
        "transflat": nc.dram_tensor("transflat", [NT * NT], FP32, kind="ExternalInput").ap(),
        "oh": nc.dram_tensor("oh", [NT, TOK], BF16, kind="ExternalInput").ap(),
        "pc": nc.dram_tensor("pc", [NT * NT, BC], FP32, kind="ExternalInput").ap(),
        "out": nc.dram_tensor("out", [1, 1], FP32, kind="ExternalOutput").ap(),
    }
    with tile.TileContext(nc) as tc:
        _emit(tc, io)
    nc.compile()
    _PROGRAM_CACHE[key] = nc
    return nc


def host_prep(inputs):
    """Build the 8 per-core input maps (host does index/layout/dtype prep)."""
    import ml_dtypes
    bf16 = ml_dtypes.bfloat16
    fp8 = ml_dtypes.float8_e4m3

    def to_fp8(x):
        return np.clip(x, -240.0, 240.0).astype(fp8)

    sent = np.asarray(inputs["sentence"]).astype(np.int64)        # [B,T]
    seq_len = np.asarray(inputs["seq_len"]).astype(np.int64)
    tags = np.asarray(inputs["tags"]).astype(np.int64)            # [B,T]
    lens = np.clip(seq_len, 1, T)
    mask = (np.arange(T)[None, :] < lens[:, None]).astype(np.float32)  # [B,T]
    embtab = np.asarray(inputs["embedding"], np.float32)

    def reorder(Wx):  # pytorch gate order i,f,g,o -> i,f,o,g
        i, f, g, o = np.split(np.asarray(Wx, np.float32), 4, 0)
        return np.concatenate([i, f, o, g], 0)

    def build_waug(W_ih, bvec, is_bwd):
        Wr = reorder(W_ih).copy()   # [4H, E]
        br = reorder(np.asarray(bvec, np.float32)[:, None])[:, 0].copy()
        Wr[3 * H:4 * H] *= 2.0      # g-gate preact x2: tanh(0.5*(2x)) = tanh(x)
        br[3 * H:4 * H] *= 2.0
        Waug = np.zeros((KAUG, G4), np.float32)
        Waug[0:E, :] = Wr.T
        Waug[E, :] = br             # ones row -> bias
        if is_bwd:
            Waug[E + 1, 0:2 * H] = -1e9  # (1-m) row -> i,f preact mask (freeze c=0)
        else:
            Waug[E + 1, 2 * H:3 * H] = -1e9  # o-gate mask: zero padded fwd outputs
        return Waug

    waug = np.concatenate(
        [build_waug(inputs["W_ih_f"], inputs["b_f"], False),
         build_waug(inputs["W_ih_b"], inputs["b_b"], True)], axis=1
    ).astype(bf16)                                               # [KAUG, 1024]

    def whh_prep(W):
        Wr = reorder(W).copy()
        Wr[0:3 * H] *= 0.5          # rhs is h~ = 2h
        return Wr.T
    whh = np.concatenate(
        [whh_prep(inputs["W_hh_f"]), whh_prep(inputs["W_hh_b"])], axis=1
    ).astype(np.float32).astype(bf16)                            # [H, 1024]
    wfc = np.ascontiguousarray(
        0.5 * np.asarray(inputs["W_fc"], np.float32).T).astype(bf16)  # [H2,NT]
    bfc = np.asarray(inputs["b_fc"], np.float32).reshape(NT, 1)
    trans = np.asarray(inputs["transitions"], np.float32)
    transT = np.ascontiguousarray(trans.T)
    transflat = np.ascontiguousarray(trans.reshape(-1))
    pstop = np.ascontiguousarray(trans[STOP, :].reshape(1, NT))

    in_maps = []
    for core in range(NCORES):
        sl = slice(core * BC, (core + 1) * BC)
        s_c, t_c, m_c = sent[sl], tags[sl], mask[sl]             # [BC,T]
        # embT: [KAUG, TOK] token-major (tok = t*BC + b);
        # embr = token blocks reversed (t -> T-1-t) for the bwd direction
        emb_g = embtab[s_c]                                      # [BC,T,E]
        embT_f = np.empty((KAUG, TOK), dtype=np.float32)
        embT_f[0:E, :] = emb_g.transpose(2, 1, 0).reshape(E, TOK)
        embT_f[E, :] = 1.0
        embT_f[E + 1, :] = 1.0 - m_c.T.reshape(-1)
        embT_m = embT_f.astype(bf16)
        embr_m = embT_f.reshape(KAUG, T, BC)[:, ::-1, :].reshape(KAUG, TOK).astype(bf16)
        # one-hot [NT, TOK]
        ohm = np.zeros((NT, TOK), np.float32)
        ttm = t_c.T.reshape(-1)
        ohm[ttm, np.arange(TOK)] = 1.0
        ohm = ohm.astype(bf16)
        # pair-count histogram [169, BC] incl STOP term
        pcm = np.zeros((NT * NT, BC), np.float32)
        text = np.concatenate([np.full((BC, 1), START, np.int64), t_c], 1)
        for b_ in range(BC):
            idx = text[b_, 1:] * NT + text[b_, :-1]
            np.add.at(pcm[:, b_], idx, 1.0)
            pcm[STOP * NT + t_c[b_, -1], b_] += 1.0
        in_maps.append({
            "embT": embT_m, "embr": embr_m, "waug": waug, "whh": whh, "wfc": wfc, "bfc": bfc,
            "transT": transT, "trans": trans, "pstop": pstop,
            "transflat": transflat, "oh": ohm, "pc": pcm,
        })
    return in_maps


def kernel(**inputs):
    nc = build_program()
    in_maps = host_prep(inputs)
    res = bass_utils.run_bass_kernel_spmd(nc, in_maps, list(range(NCORES)))
    total = sum(float(r["out"][0, 0]) for r in res.results)
    return np.float32(total / B)


# revision 16
# speedup vs baseline: 1.3266x; 1.1104x over previous
"""BiLSTM-CRF NLL loss kernel for 8 Trainium2 NeuronCores (pure batch data-parallel).

Self-contained: hardcodes all shapes. Strategy per core (32 of 256 sequences):
  1. Host pre-gathers + transposes + augments the embeddings into embT
     [KAUG, TOK] bf16 (token-major, rows = emb dims + ones + (1-m)), streamed
     into SBUF end-ranges-first so the LSTM can start immediately.
  2. gx = Waug @ embT is computed by the PE directly INTO the per-step gates
     PSUM banks (ring of 2-step banks per direction, gate-major layout),
     a few steps ahead of the recurrence; the W_hh matmuls accumulate on
     top. No identity inject, no PSUM->SBUF copies.
  3. Two per-direction dependency chains (fwd t ascending, bwd descending),
     gates via tanh only (sigmoid(x)=0.5*tanh(x/2)+0.5), all loop DVE ops in
     bf16 (2x mode). Cell slot stores c2 = 2c; h~ = 2h with W_hh, W_fc
     pre-halved on host.
  4. fc GEMM + exp per 512-token chunk as soon as both h halves exist,
     interleaved into the loop's engine slack.
  5. CRF split in the middle: alpha runs t=0..95 forward, beta runs
     t=191..96 backward (invariant beta^T alpha), both chains concurrent,
     in scaled probability space (P scaled by e^-CCENT, renorm every 32).
  6. Gold score from host-built histograms/one-hots via PE reductions.
     Output: per-core sum of (fwd - gold); host divides by B.
"""

import numpy as np

import concourse.bacc as bacc
import concourse.bass as bass
import concourse.mybir as mybir
import concourse.tile as tile
from concourse import bass_utils

B, T, E, H2, V, NT = 256, 192, 300, 256, 11626, 13
H = H2 // 2          # 128
G4 = 4 * H           # 512
START, STOP = 0, 10
NCORES = 8
BC = B // NCORES     # 32 sequences per core
TOK = BC * T         # 6144 tokens per core, t-major: tok = t*BC + b
KAUG = E + 2         # emb dims + ones row + (1-m) row
KCH = [(0, 128), (128, 256), (256, KAUG)]   # K chunks of augmented GEMM
KPAIR = KAUG // 2    # 151 fp8 DoubleRow pairs
KCHD = [(0, 128), (128, KPAIR)]             # pair-space chunks for gx
RENORM = 32          # CRF renorm period
CCENT = 3.0
HM = T // 2          # 96: CRF meeting point
NREN = 2 * (HM // RENORM - 1)  # renorm events total (alpha + beta)
NCH = 12             # fc chunks (512 tokens each)
FCW = TOK // NCH     # 512
WS = 8               # steps per gates PSUM bank window

FP32 = mybir.dt.float32
BF16 = mybir.dt.bfloat16
FP8 = mybir.dt.float8e4
DR = mybir.MatmulPerfMode.DoubleRow
AF = mybir.ActivationFunctionType
ALU = mybir.AluOpType

_PROGRAM_CACHE = {}


def _emit(tc, io):
    nc = tc.nc
    embT = io["embT"]; waug = io["waug"]; whh = io["whh"]
    wfc = io["wfc"]; bfc = io["bfc"]
    transT = io["transT"]; trans = io["trans"]; pstop = io["pstop"]
    transflat = io["transflat"]
    oh = io["oh"]; pc = io["pc"]; out = io["out"]

    import contextlib
    ctx = contextlib.ExitStack()
    with ctx:
        consts = ctx.enter_context(tc.tile_pool(name="consts", bufs=1))

        # ---------- embT streamed in; embr = token blocks reversed ----------
        embT_sb = [consts.tile([k1 - k0, TOK], BF16, name=f"embT{i}")
                   for i, (k0, k1) in enumerate(KCH)]
        embR_sb = [consts.tile([k1 - k0, TOK], BF16, name=f"embR{i}")
                   for i, (k0, k1) in enumerate(KCH)]
        embr = io["embr"]
        NBLK = 4
        BW = TOK // NBLK  # 1536 tokens = 48 steps per superblock
        eng_ring = [nc.sync, nc.scalar, nc.gpsimd]
        ei = 0
        for bi in range(NBLK):
            c0, c1 = bi * BW, (bi + 1) * BW
            for ki, (k0, k1) in enumerate(KCH):
                eng = eng_ring[ei % 3]; ei += 1
                eng.dma_start(out=embT_sb[ki][:, c0:c1], in_=embT[k0:k1, c0:c1])
                eng = eng_ring[ei % 3]; ei += 1
                eng.dma_start(out=embR_sb[ki][:, c0:c1], in_=embr[k0:k1, c0:c1])

        # ---------- small constants ----------
        waug_sb = [consts.tile([k1 - k0, 2 * G4], BF16, name=f"waug{i}")
                   for i, (k0, k1) in enumerate(KCH)]
        for (k0, k1), t_ in zip(KCH, waug_sb):
            nc.sync.dma_start(out=t_[:], in_=waug[k0:k1, :])
        whh_sb = consts.tile([H, 2 * G4], BF16)
        nc.sync.dma_start(out=whh_sb[:], in_=whh[:])
        wfc_sb = consts.tile([H, 2 * NT], BF16)
        nc.sync.dma_start(out=wfc_sb[:, 0:NT], in_=wfc[0:H, :])
        nc.sync.dma_start(out=wfc_sb[:, NT:2 * NT], in_=wfc[H:H2, :])
        bfc_sb = consts.tile([NT, 1], FP32)
        nc.sync.dma_start(out=bfc_sb[:], in_=bfc[:])
        transT_sb = consts.tile([NT, NT], FP32)
        nc.sync.dma_start(out=transT_sb[:], in_=transT[:])
        trans_sb = consts.tile([NT, NT], FP32)
        nc.sync.dma_start(out=trans_sb[:], in_=trans[:])
        pstop_sb = consts.tile([1, NT], FP32)
        nc.sync.dma_start(out=pstop_sb[:], in_=pstop[:])
        tf_sb = consts.tile([128, 2], FP32)   # transflat split columns
        tfa = transflat[0:128]
        nc.gpsimd.dma_start(out=tf_sb[:, 0:1],
                            in_=bass.AP(tensor=tfa.tensor, offset=tfa.offset, ap=[[1, 128], [1, 1]]))
        tfb = transflat[128:169]
        nc.gpsimd.dma_start(out=tf_sb[0:41, 1:2],
                            in_=bass.AP(tensor=tfb.tensor, offset=tfb.offset, ap=[[1, 41], [1, 1]]))
        pc_sb = consts.tile([128, 2 * BC], FP32)
        nc.gpsimd.dma_start(out=pc_sb[:, 0:BC], in_=pc[0:128, :])
        nc.gpsimd.dma_start(out=pc_sb[0:41, BC:2 * BC], in_=pc[128:169, :])
        ones13_sb = consts.tile([NT, 1], FP32)
        nc.vector.memset(ones13_sb[:], 1.0)
        onesrow_sb = consts.tile([1, NT], FP32)
        nc.vector.memset(onesrow_sb[:], 1.0)
        negc_sb = consts.tile([NT, 1], FP32)
        nc.vector.memset(negc_sb[:], -CCENT)
        # alpha stationary: exp(transT - C); lhsT=exp(transT) -> applies P
        pts_sb = consts.tile([NT, NT], BF16)
        nc.scalar.activation(pts_sb[:], transT_sb[:], AF.Exp, bias=negc_sb[:, 0:1])
        # beta stationary: exp(trans - C); lhsT = P_s -> applies P_s^T
        pbs_sb = consts.tile([NT, NT], BF16)
        nc.scalar.activation(pbs_sb[:], trans_sb[:], AF.Exp, bias=negc_sb[:, 0:1])
        ones13b_sb = consts.tile([NT, 1], BF16)
        nc.vector.memset(ones13b_sb[:], 1.0)
        # pstop row exp (unscaled)
        pse_sb = consts.tile([1, NT], FP32)
        nc.scalar.activation(pse_sb[:], pstop_sb[:], AF.Exp)
        onescol_sb = consts.tile([1, BC], FP32)
        nc.vector.memset(onescol_sb[:], 1.0)

        # ---------- persistent loop state ----------
        hallp = ctx.enter_context(tc.tile_pool(name="hallp", bufs=1))
        h_all = [hallp.tile([H, TOK], BF16, name=f"hall{d}") for d in range(2)]
        late = ctx.enter_context(tc.tile_pool(name="late", bufs=1))
        ef = late.tile([NT, TOK], BF16)
        oh_sb = late.tile([NT, TOK], BF16)
        em_acc = [late.tile([NT, BC], FP32, name=f"emacc{i}") for i in range(2)]
        zbuf = late.tile([1, NREN * BC], FP32)
        nc.vector.memset(em_acc[0][:], 0.0)
        nc.gpsimd.dma_start(out=oh_sb[:], in_=oh[:])

        hinit = consts.tile([H, BC], BF16)
        nc.vector.memset(hinit[:], 0.0)
        emtp = ctx.enter_context(tc.tile_pool(name="emtp", bufs=1))
        emqp = ctx.enter_context(tc.tile_pool(name="emqp", bufs=2))

        # gates PSUM ring: per dir, 2 banks x [128, 4 gates x 8 steps x 32]
        # fp32 (gate-major: gate g at cols [g*8BC + (s%8)*BC]); all 8 PSUM
        # banks are used during the loop (fc runs post-loop).
        gbank = tc.alloc_tile_pool(name="gbank", bufs=2, space="PSUM")
        banks = [[gbank.tile([H, WS * 4 * BC], FP32, name=f"bank{d}{w}", tag=f"bank{d}")
                  for w in range(2)] for d in range(2)]
        # T tiles: ring of 2; each ring tile holds both dirs at base d*5BC
        # with per-dir layout [Ti|Tf|To|Tg|c2], so q/c2 stay per-dir
        # contiguous while the cell tanh merges both dirs in one ACT op.
        tpool = tc.alloc_tile_pool(name="ltt", bufs=1)
        ttiles = [tpool.tile([H, 2 * 5 * BC], BF16, name=f"tt{r}") for r in range(2)]
        for d in range(2):
            nc.vector.memset(ttiles[0][:, d * 5 * BC + 4 * BC:(d + 1) * 5 * BC], 0.0)
        qpool = tc.alloc_tile_pool(name="lq", bufs=2)
        tcpool = tc.alloc_tile_pool(name="ltc", bufs=2)

        # ---------- gx emission (direct to PSUM banks) ----------
        def emit_gx1(d, w, g, ki):
            """One gx matmul: window w (steps 8w..8w+7), dir d, gate g, kchunk ki."""
            bk = banks[d][w % 2]
            k0, k1 = KCH[ki]
            kw = k1 - k0
            lhsT = waug_sb[ki][0:kw, d * G4 + g * H: d * G4 + (g + 1) * H]
            esb = (embT_sb if d == 0 else embR_sb)[ki]
            rhs = esb[0:kw, 8 * w * BC:(8 * w + WS) * BC]
            nc.tensor.matmul(bk[:, g * WS * BC:(g + 1) * WS * BC],
                             lhsT, rhs,
                             start=(ki == 0), stop=False,
                             skip_group_check=True)

        GX_ITEMS = [(d, g, ki) for d in range(2) for g in range(4)
                    for ki in range(3)]  # 24 per window, 3 per step

        # ---------- fc/emit ops (executed post-loop) ----------
        fc_state = {"nacc": 0}

        def fc_ops(k):
            cols = slice(k * FCW, (k + 1) * FCW)
            st = {}

            def op1():
                st["mm"] = fcp.tile([NT, FCW], FP32, tag="fcmm", name="fcmm")
                nc.tensor.matmul(st["mm"][:], wfc_sb[:, 0:NT],
                                 h_all[0][:, cols], start=True, stop=False,
                                 skip_group_check=True)

            def op2():
                nc.tensor.matmul(st["mm"][:], wfc_sb[:, NT:2 * NT],
                                 h_all[1][:, cols], start=False, stop=True,
                                 skip_group_check=True)

            def op3():
                c0 = k * FCW
                nc.scalar.activation(ef[:, c0:c0 + FCW // 2],
                                     st["mm"][:, 0:FCW // 2], AF.Exp,
                                     bias=bfc_sb[:, 0:1])

            def op4():
                c0 = k * FCW
                nc.scalar.activation(ef[:, c0 + FCW // 2:c0 + FCW],
                                     st["mm"][:, FCW // 2:FCW], AF.Exp,
                                     bias=bfc_sb[:, 0:1])

            def op5():
                st["emtmp"] = emtp.tile([NT, FCW], BF16, tag="emtmp", name="emtmp")
                nc.vector.scalar_tensor_tensor(
                    st["emtmp"][:], st["mm"][:], bfc_sb[:, 0:1], oh_sb[:, cols],
                    ALU.add, ALU.mult)

            def op6():
                st["emq"] = emqp.tile([NT, BC], FP32, tag="emq", name="emq")
                nc.vector.reduce_sum(
                    st["emq"][:],
                    st["emtmp"][:].rearrange("p (t b) -> p b t", t=FCW // BC),
                    axis=mybir.AxisListType.X)

            def op7():
                i = fc_state["nacc"]
                nc.gpsimd.tensor_tensor(em_acc[(i + 1) % 2][:], em_acc[i % 2][:],
                                        st["emq"][:], ALU.add)
                fc_state["nacc"] = i + 1

            return [op1, op2, op3, op4, op5, op6, op7]

        # ---------- LSTM step ----------
        def step_mms(d, s, h_prev_ap):
            bk = banks[d][(s // WS) % 2]
            par = s % WS
            for g in range(4):
                nc.tensor.matmul(
                    bk[:, g * WS * BC + par * BC: g * WS * BC + (par + 1) * BC],
                    whh_sb[:, d * G4 + g * H: d * G4 + (g + 1) * H],
                    h_prev_ap,
                    start=False, stop=True, skip_group_check=True)

        def step_gates(d, s):
            """per-dir gates tanh + q + c2'."""
            bk = banks[d][(s // WS) % 2]
            par = s % WS
            b0 = d * 5 * BC
            gates_ap = bass.AP(tensor=bk.tensor, offset=bk[:, par * BC].offset,
                               ap=[bk.ap[0], [WS * BC, 4], [1, BC]])
            tt = ttiles[s % 2]
            tnx = ttiles[(s + 1) % 2]
            nc.scalar.activation(tt[:, b0:b0 + 4 * BC], gates_ap, AF.Tanh, scale=0.5)
            q = qpool.tile([H, 2 * BC], BF16, tag=f"q{d}", name=f"q{d}")
            nc.vector.scalar_tensor_tensor(
                q[:], tt[:, b0:b0 + 2 * BC], 1.0, tt[:, b0 + 3 * BC:b0 + 5 * BC],
                ALU.add, ALU.mult)
            nc.vector.scalar_tensor_tensor(
                tnx[:, b0 + 4 * BC:b0 + 5 * BC], q[:, BC:2 * BC], 0.5, q[:, 0:BC],
                ALU.mult, ALU.add)

        def step_cell(d, s):
            """per-dir cell tanh."""
            tnx = ttiles[(s + 1) % 2]
            b0 = d * 5 * BC
            tc_ = tcpool.tile([H, BC], BF16, tag=f"tc{d}", name=f"tc{d}")
            nc.scalar.activation(tc_[:], tnx[:, b0 + 4 * BC:b0 + 5 * BC],
                                 AF.Tanh, scale=0.5)
            return tc_

        def step_h(d, s, t, tc_):
            tt = ttiles[s % 2]
            b0 = d * 5 * BC
            hdst = h_all[d][:, t * BC:(t + 1) * BC]
            nc.vector.scalar_tensor_tensor(
                hdst, tt[:, b0 + 2 * BC:b0 + 3 * BC], 1.0, tc_[:],
                ALU.add, ALU.mult)
            return hdst

        # ---------- main loop ----------
        # prologue: window 0 only; window w streams in during window w-1
        # (ring of 2 banks, 1-step WAR slack at boundaries)
        for (d, g, ki) in GX_ITEMS:
            emit_gx1(d, 0, g, ki)
        hp = [hinit[:], hinit[:]]
        for s in range(T):
            step_mms(0, s, hp[0])
            step_mms(1, s, hp[1])
            step_gates(0, s)
            tc0 = step_cell(0, s)
            hp[0] = step_h(0, s, s, tc0)
            step_gates(1, s)
            tc1 = step_cell(1, s)
            hp[1] = step_h(1, s, T - 1 - s, tc1)
            w, j = s // WS + 1, s % WS
            if w < T // WS:
                for (d, g, ki) in GX_ITEMS[3 * j:3 * j + 3]:
                    emit_gx1(d, w, g, ki)

        tcpool.release()
        qpool.release()
        tpool.release()
        gbank.release()

        # ---------- fc + emit (post-loop, overlaps the CRF phase) ----------
        fcp = tc.alloc_tile_pool(name="fcp", bufs=2, space="PSUM")
        order = []
        lo_, hi_ = 0, NCH - 1
        while lo_ <= hi_:
            order.append(lo_); lo_ += 1
            if lo_ <= hi_:
                order.append(hi_); hi_ -= 1
        for k in order:
            for op in fc_ops(k):
                op()

        # ---------- CRF: alpha fwd 0..95, beta bwd 191..96 ----------
        apool = ctx.enter_context(tc.tile_pool(name="apool", bufs=3))
        bpool = ctx.enter_context(tc.tile_pool(name="bpool", bufs=3))
        crfp = tc.alloc_tile_pool(name="crfp", bufs=2, space="PSUM")
        crfz = tc.alloc_tile_pool(name="crfz", bufs=1, space="PSUM")
        zr = ctx.enter_context(tc.tile_pool(name="zr", bufs=2))
        efx = ctx.enter_context(tc.tile_pool(name="efx", bufs=2))

        A = apool.tile([NT, BC], BF16, tag="A", name="A")
        nc.vector.memset(A[:], 0.0)
        nc.vector.memset(A[START:START + 1, :], 1.0)
        # beta init: pstop broadcast over batch: pse^T @ ones[1,BC]
        Bt = crfp.tile([NT, BC], FP32, tag="rb", name="rb")
        nc.tensor.matmul(Bt[:], pse_sb[:], onescol_sb[:], start=True, stop=True,
                         skip_group_check=True)

        pend = [None, None]  # pre-scaled ef slices after renorm [alpha, beta]
        nren = [0]

        def renorm(X, which, tnext):
            """Fold 1/Z into the next step's ef slice; record Z. X must be SBUF."""
            k = nren[0]; nren[0] += 1
            zrow = crfz.tile([1, BC], FP32, tag="zrow", name="zrow")
            nc.tensor.matmul(zrow[:], ones13b_sb[:], X[:], start=True, stop=True,
                             skip_group_check=True)
            nc.scalar.copy(zbuf[:, k * BC:(k + 1) * BC], zrow[:])
            zrec = zr.tile([1, BC], FP32, tag=f"zrec{which}", name="zrec")
            nc.vector.reciprocal(zrec[:], zrow[:])
            zbc = crfz.tile([NT, BC], FP32, tag="zbc", name="zbc")
            nc.tensor.matmul(zbc[:], onesrow_sb[:], zrec[:], start=True, stop=True,
                             skip_group_check=True)
            nxt = efx.tile([NT, BC], BF16, tag=f"efx{which}", name="efx")
            cols = slice(tnext * BC, (tnext + 1) * BC)
            nc.vector.tensor_tensor(nxt[:], ef[:, cols], zbc[:], ALU.mult)
            pend[which] = nxt

        for i in range(HM):
            ta = i           # alpha consumes ef[ta]
            tb = T - 1 - i   # beta consumes ef[tb]
            # alpha: A <- (P@A) * ef[ta]
            r = crfp.tile([NT, BC], FP32, tag="ra", name="ra")
            nc.tensor.matmul(r[:], pts_sb[:], A[:], start=True, stop=True,
                             skip_group_check=True)
            A2 = apool.tile([NT, BC], BF16, tag="A", name="A")
            ef_ap = pend[0][:] if pend[0] is not None else ef[:, ta * BC:(ta + 1) * BC]
            pend[0] = None
            nc.vector.tensor_tensor(A2[:], r[:], ef_ap, ALU.mult)
            A = A2
            if (i + 1) % RENORM == 0 and i + 1 < HM:
                renorm(A, 0, ta + 1)
            # beta: B <- P^T @ (ef[tb] * B)   (Bt lives in PSUM except renorms)
            X = bpool.tile([NT, BC], BF16, tag="X", name="X")
            ef_bp = pend[1][:] if pend[1] is not None else ef[:, tb * BC:(tb + 1) * BC]
            pend[1] = None
            nc.vector.tensor_tensor(X[:], Bt[:], ef_bp, ALU.mult)
            B2 = crfp.tile([NT, BC], FP32, tag="rb", name="rb")
            nc.tensor.matmul(B2[:], pbs_sb[:], X[:], start=True, stop=True,
                             skip_group_check=True)
            Bt = B2
            if (i + 1) % RENORM == 0 and i + 1 < HM:
                Bs = bpool.tile([NT, BC], BF16, tag="Bs", name="Bs")
                nc.vector.tensor_copy(Bs[:], B2[:])
                Bt = Bs
                renorm(Bt, 1, tb - 1)

        # ---------- finals ----------
        fin = ctx.enter_context(tc.tile_pool(name="fin", bufs=1))
        # meet: fwd = ln(sum_j A[j]*B[j]) + sum ln Z
        meet = fin.tile([NT, BC], FP32)
        nc.vector.tensor_tensor(meet[:], A[:], Bt[:], ALU.mult)
        crfz.release()
        crfp.release()
        fcp.release()
        finp = ctx.enter_context(tc.tile_pool(name="finp", bufs=1, space="PSUM"))
        emred = em_acc[fc_state["nacc"] % 2]
        gold = finp.tile([1, BC], FP32)
        nc.tensor.matmul(gold[:], tf_sb[:, 0:1], pc_sb[:, 0:BC], start=True, stop=False,
                         skip_group_check=True)
        nc.tensor.matmul(gold[:], tf_sb[0:41, 1:2], pc_sb[0:41, BC:2 * BC],
                         start=False, stop=False, skip_group_check=True)
        nc.tensor.matmul(gold[:], ones13_sb[:], emred[:], start=False, stop=True,
                         skip_group_check=True)
        fmm = finp.tile([1, BC], FP32)
        nc.tensor.matmul(fmm[:], ones13_sb[:], meet[:], start=True, stop=True,
                         skip_group_check=True)
        lnz = fin.tile([1, NREN * BC], FP32)
        nc.scalar.activation(lnz[:], zbuf[:], AF.Ln)
        lsum = fin.tile([1, BC], FP32)
        nc.vector.reduce_sum(
            lsum[:], lnz[:].rearrange("p (k b) -> p b k", k=NREN),
            axis=mybir.AxisListType.X)
        lfin = fin.tile([1, BC], FP32)
        nc.scalar.activation(lfin[:], fmm[:], AF.Ln)
        fwd = fin.tile([1, BC], FP32)
        nc.vector.tensor_tensor(fwd[:], lfin[:], lsum[:], ALU.add)
        nll = fin.tile([1, BC], FP32)
        nc.vector.tensor_tensor(nll[:], fwd[:], gold[:], ALU.subtract)
        nllc = fin.tile([1, BC], FP32)
        nc.vector.tensor_scalar_add(nllc[:], nll[:], CCENT * T)
        tot = fin.tile([1, 1], FP32)
        nc.vector.reduce_sum(tot[:], nllc[:], axis=mybir.AxisListType.X)
        nc.sync.dma_start(out=out[:], in_=tot[:])


def build_program():
    key = "nc"
    if key in _PROGRAM_CACHE:
        return _PROGRAM_CACHE[key]
    nc = bacc.Bacc("TRN2", target_bir_lowering=False, debug=False, num_devices=NCORES)
    io = {
        "embT": nc.dram_tensor("embT", [KAUG, TOK], BF16, kind="ExternalInput").ap(),
        "embr": nc.dram_tensor("embr", [KAUG, TOK], BF16, kind="ExternalInput").ap(),
        "waug": nc.dram_tensor("waug", [KAUG, 2 * G4], BF16, kind="ExternalInput").ap(),
        "whh": nc.dram_tensor("whh", [H, 2 * G4], BF16, kind="ExternalInput").ap(),
        "wfc": nc.dram_tensor("wfc", [H2, NT], BF16, kind="ExternalInput").ap(),
        "bfc": nc.dram_tensor("bfc", [NT, 1], FP32, kind="ExternalInput").ap(),
        "transT": nc.dram_tensor("transT", [NT, NT], FP32, kind="ExternalInput").ap(),
        "trans": nc.dram_tensor("trans", [NT, NT], FP32, kind="ExternalInput").ap(),
        "pstop": nc.dram_tensor("pstop", [1, NT], FP32, kind="ExternalInput").ap(),
        "transflat": nc.dram_tensor("transflat", [NT * NT], FP32, kind="ExternalInput").ap(),
        "oh": nc.dram_tensor("oh", [NT, TOK], BF16, kind="ExternalInput").ap(),
        "pc": nc.dram_tensor("pc", [NT * NT, BC], FP32, kind="ExternalInput").ap(),
        "out": nc.dram_tensor("out", [1, 1], FP32, kind="ExternalOutput").ap(),
    }
    with tile.TileContext(nc) as tc:
        _emit(tc, io)
    nc.compile()
    _PROGRAM_CACHE[key] = nc
    return nc


def host_prep(inputs):
    """Build the 8 per-core input maps (host does index/layout/dtype prep)."""
    import ml_dtypes
    bf16 = ml_dtypes.bfloat16
    fp8 = ml_dtypes.float8_e4m3

    def to_fp8(x):
        return np.clip(x, -240.0, 240.0).astype(fp8)

    sent = np.asarray(inputs["sentence"]).astype(np.int64)        # [B,T]
    seq_len = np.asarray(inputs["seq_len"]).astype(np.int64)
    tags = np.asarray(inputs["tags"]).astype(np.int64)            # [B,T]
    lens = np.clip(seq_len, 1, T)
    mask = (np.arange(T)[None, :] < lens[:, None]).astype(np.float32)  # [B,T]
    embtab = np.asarray(inputs["embedding"], np.float32)

    def reorder(Wx):  # pytorch gate order i,f,g,o -> i,f,o,g
        i, f, g, o = np.split(np.asarray(Wx, np.float32), 4, 0)
        return np.concatenate([i, f, o, g], 0)

    def build_waug(W_ih, bvec, is_bwd):
        Wr = reorder(W_ih).copy()   # [4H, E]
        br = reorder(np.asarray(bvec, np.float32)[:, None])[:, 0].copy()
        Wr[3 * H:4 * H] *= 2.0      # g-gate preact x2: tanh(0.5*(2x)) = tanh(x)
        br[3 * H:4 * H] *= 2.0
        Waug = np.zeros((KAUG, G4), np.float32)
        Waug[0:E, :] = Wr.T
        Waug[E, :] = br             # ones row -> bias
        if is_bwd:
            Waug[E + 1, 0:2 * H] = -1e9  # (1-m) row -> i,f preact mask (freeze c=0)
        else:
            Waug[E + 1, 2 * H:3 * H] = -1e9  # o-gate mask: zero padded fwd outputs
        return Waug

    waug = np.concatenate(
        [build_waug(inputs["W_ih_f"], inputs["b_f"], False),
         build_waug(inputs["W_ih_b"], inputs["b_b"], True)], axis=1
    ).astype(bf16)                                               # [KAUG, 1024]

    def whh_prep(W):
        Wr = reorder(W).copy()
        Wr[0:3 * H] *= 0.5          # rhs is h~ = 2h
        return Wr.T
    whh = np.concatenate(
        [whh_prep(inputs["W_hh_f"]), whh_prep(inputs["W_hh_b"])], axis=1
    ).astype(np.float32).astype(bf16)                            # [H, 1024]
    wfc = np.ascontiguousarray(
        0.5 * np.asarray(inputs["W_fc"], np.float32).T).astype(bf16)  # [H2,NT]
    bfc = np.asarray(inputs["b_fc"], np.float32).reshape(NT, 1)
    trans = np.asarray(inputs["transitions"], np.float32)
    transT = np.ascontiguousarray(trans.T)
    transflat = np.ascontiguousarray(trans.reshape(-1))
    pstop = np.ascontiguousarray(trans[STOP, :].reshape(1, NT))

    in_maps = []
    for core in range(NCORES):
        sl = slice(core * BC, (core + 1) * BC)
        s_c, t_c, m_c = sent[sl], tags[sl], mask[sl]             # [BC,T]
        # embT: [KAUG, TOK] token-major (tok = t*BC + b);
        # embr = token blocks reversed (t -> T-1-t) for the bwd direction
        emb_g = embtab[s_c]                                      # [BC,T,E]
        embT_f = np.empty((KAUG, TOK), dtype=np.float32)
        embT_f[0:E, :] = emb_g.transpose(2, 1, 0).reshape(E, TOK)
        embT_f[E, :] = 1.0
        embT_f[E + 1, :] = 1.0 - m_c.T.reshape(-1)
        embT_m = embT_f.astype(bf16)
        embr_m = embT_f.reshape(KAUG, T, BC)[:, ::-1, :].reshape(KAUG, TOK).astype(bf16)
        # one-hot [NT, TOK]
        ohm = np.zeros((NT, TOK), np.float32)
        ttm = t_c.T.reshape(-1)
        ohm[ttm, np.arange(TOK)] = 1.0
        ohm = ohm.astype(bf16)
        # pair-count histogram [169, BC] incl STOP term
        pcm = np.zeros((NT * NT, BC), np.float32)
        text = np.concatenate([np.full((BC, 1), START, np.int64), t_c], 1)
        for b_ in range(BC):
            idx = text[b_, 1:] * NT + text[b_, :-1]
            np.add.at(pcm[:, b_], idx, 1.0)
            pcm[STOP * NT + t_c[b_, -1], b_] += 1.0
        in_maps.append({
            "embT": embT_m, "embr": embr_m, "waug": waug, "whh": whh, "wfc": wfc, "bfc": bfc,
            "transT": transT, "trans": trans, "pstop": pstop,
            "transflat": transflat, "oh": ohm, "pc": pcm,
        })
    return in_maps


def kernel(**inputs):
    nc = build_program()
    in_maps = host_prep(inputs)
    res = bass_utils.run_bass_kernel_spmd(nc, in_maps, list(range(NCORES)))
    total = sum(float(r["out"][0, 0]) for r in res.results)
    return np.float32(total / B)
